# revision 1
# baseline (speedup 1.0000x reference)
"""Trainium2 Bass kernel for a 2-layer GCN (DGL GraphConv, norm='both').

Reference computation (per layer):
    h = relu( deg_in^-0.5 * segment_sum( ((x * deg_out^-0.5) @ W)[src], dst ) + b )
then logits = h2 @ Wc + bc.

Distribution: nodes are relabeled into 128-wide blocks, blocks are
load-balanced across the 8 NeuronCores (snake assignment by edge count),
giving every core an equal, structurally identical workload (SPMD: one
program, per-core data). Per layer:
  stage A: each core computes g = (x @ W) * s_out for its node shard
  AllGather: g shards -> full g table in every core's DRAM
  stage B: blocks are processed in groups of 4; per group, edge messages
    are gathered row-wise from the g table with one dma_gather per int16
    sub-table chunk; the per-block segment-sum is one-hot x messages
    matmuls accumulated in PSUM; epilogue scales by s_in, transposes,
    adds bias, relu -> h^T kept in SBUF.
Layer 2's epilogue is fused with the classifier: logits = h2 @ Wc + bc,
written per shard; the host reassembles and inverse-permutes.

All index preprocessing (degree counts, edge sorting/padding, relabeling)
is host-side numpy on integer graph structure; float math is on device.
"""
import math
from dataclasses import dataclass

import numpy as np

import concourse.bacc as bacc
import concourse.mybir as mybir
import concourse.tile as tile
from concourse.bass_utils import run_bass_kernel_spmd

f32 = mybir.dt.float32
bf16 = mybir.dt.bfloat16
i16 = mybir.dt.int16

P = 128  # partitions / node block size

# numpy view of bfloat16 for host-side constant/input arrays
import ml_dtypes  # noqa: E402  (ships with jax)

np_bf16 = ml_dtypes.bfloat16


@dataclass
class Cfg:
    n_nodes: int = 100000
    in_feats: int = 128
    num_classes: int = 4
    n_cores: int = 8
    nb: int = 98          # node blocks per core
    chunk: int = 25088    # gather sub-table rows (int16-addressable)
    group: int = 2        # blocks per gather group

    @property
    def npc(self):        # nodes per core
        return self.nb * P

    @property
    def npad(self):       # padded node count
        return self.n_cores * self.npc

    @property
    def n_chunks(self):
        return math.ceil(self.npad / self.chunk)

    @property
    def n_groups(self):
        return math.ceil(self.nb / self.group)


CFG = Cfg()


class Geometry:
    """Static slot layout derived from per-(block position, chunk) subtile
    capacities. Identical across cores (capacities are cross-core maxima)."""

    def __init__(self, cfg: Cfg, cap: np.ndarray):
        self.cap = cap  # [nb, n_chunks]
        G, NG, NCH = cfg.group, cfg.n_groups, cfg.n_chunks
        self.grp_blocks = [list(range(g * G, min((g + 1) * G, cfg.nb)))
                           for g in range(NG)]
        # per group: subtile base of (chunk, block-in-group), chunk ranges
        self.sub_base = []   # [NG][NCH][len(grp)] -> subtile index
        self.chunk_rng = []  # [NG][NCH] -> (s0, s1)
        self.Rg = []         # [NG] total subtiles
        for g in range(NG):
            blocks = self.grp_blocks[g]
            base = []
            rng = []
            s = 0
            for c in range(NCH):
                row = []
                c0 = s
                for b in blocks:
                    row.append(s)
                    s += int(cap[b, c])
                base.append(row)
                rng.append((c0, s))
            self.sub_base.append(base)
            self.chunk_rng.append(rng)
            self.Rg.append(s)
        self.Rmax = max(self.Rg)
        self.capmax = int(cap.max())
        # per block: ordered list of (group-subtile start, count) segments
        self.blk_segs = [[] for _ in range(cfg.nb)]
        for g in range(NG):
            for c in range(NCH):
                for i, b in enumerate(self.grp_blocks[g]):
                    n = int(cap[b, c])
                    if n:
                        self.blk_segs[b].append((self.sub_base[g][c][i], n))


def preprocess(cfg: Cfg, src: np.ndarray, dst: np.ndarray):
    """Relabel nodes, sort/pad edges into grouped gather metadata.

    Returns (geom, node_new, idx16, loc):
      idx16[m]: [n_groups, P, Rmax*8] int16 indices (16-wrapped, 8x repl)
      loc[m]:   [n_groups, P, Rmax]  f32 local dst in 0..127, 1000 for pads
    Group slot j -> partition j%128, group subtile j//128.
    """
    ncores, nb, nch, CH = cfg.n_cores, cfg.nb, cfg.n_chunks, cfg.chunk
    G = cfg.group
    n_blocks = ncores * nb

    # block load balancing: snake-assign blocks by edge count
    blk_tot = np.bincount(dst >> 7, minlength=n_blocks)
    order = np.argsort(-blk_tot, kind="stable")
    rank = np.arange(n_blocks)
    lane = rank % ncores
    rev = (rank // ncores) % 2 == 1
    core_of_rank = np.where(rev, ncores - 1 - lane, lane)
    core_of_old = np.empty(n_blocks, np.int64)
    pos_of_old = np.empty(n_blocks, np.int64)
    core_of_old[order] = core_of_rank
    pos_of_old[order] = rank // ncores
    new_blk_of_old = core_of_old * nb + pos_of_old
    node_ar = np.arange(cfg.npad, dtype=np.int64)
    node_new = new_blk_of_old[node_ar >> 7] * P + (node_ar & 127)

    src_n = node_new[src]
    dst_n = node_new[dst]

    blk = dst_n >> 7                      # new global block
    m_arr = blk // nb
    b_arr = blk % nb                      # position within core
    chunk_id = src_n // CH
    # sorted by (core, block, chunk, src): key-contiguous for `within`
    perm = np.lexsort((src_n, chunk_id, blk))
    # re-derive per-edge fields in sorted order
    src_s = src_n[perm]
    m_s = m_arr[perm]
    b_s = b_arr[perm]
    g_s = b_s // G
    c_s = chunk_id[perm]
    loc_s = (dst_n[perm] & 127).astype(np.float32)

    key = (m_s * nb + b_s) * nch + c_s
    counts = np.bincount(key, minlength=n_blocks * nch).reshape(
        ncores, nb, nch)
    cap = (-(-counts // P)).max(axis=0)   # [nb, nch] cross-core capacity
    geom = Geometry(cfg, cap)

    # slot of each edge within its group
    sub_base_arr = np.zeros((nb, nch), np.int64)
    for g in range(cfg.n_groups):
        for c in range(nch):
            for i, b in enumerate(geom.grp_blocks[g]):
                sub_base_arr[b, c] = geom.sub_base[g][c][i]
    ecum = np.zeros(n_blocks * nch + 1, np.int64)
    np.cumsum(counts.reshape(-1), out=ecum[1:])
    within = np.arange(len(src_s)) - ecum[key]
    slot = sub_base_arr[b_s, c_s] * P + within

    Rmax = geom.Rmax
    loc = np.full((ncores, cfg.n_groups, P, Rmax), 1000.0, np_bf16)
    t_arr = slot // P
    p_arr = slot % P
    loc[m_s, g_s, p_arr, t_arr] = loc_s.astype(np_bf16)

    val = (src_s - c_s * CH).astype(np.int16)
    c16 = slot // 16
    r16 = slot % 16
    flat = np.zeros((ncores, cfg.n_groups, 16, Rmax * 8), np.int16)
    flat[m_s, g_s, r16, c16] = val
    idx16 = np.tile(flat, (1, 1, 8, 1))
    return geom, node_new, idx16, loc


def build_program(cfg: Cfg, geom: Geometry, single_core_sim=False):
    F = cfg.in_feats
    NB, NPC, NPAD = cfg.nb, cfg.npc, cfg.npad
    NCH, CH, NG = cfg.n_chunks, cfg.chunk, cfg.n_groups
    NCLS = cfg.num_classes
    Rmax, capmax = geom.Rmax, geom.capmax

    n_dev = 1 if single_core_sim else cfg.n_cores
    nc = bacc.Bacc("TRN2", target_bir_lowering=False, debug=False,
                   num_devices=n_dev)

    xT = nc.declare_dram_parameter("xT", [F, NPC], f32, isOutput=False)
    W1 = nc.declare_dram_parameter("W1", [F, F], f32, isOutput=False)
    W2 = nc.declare_dram_parameter("W2", [F, F], f32, isOutput=False)
    Wc = nc.declare_dram_parameter("Wc", [F, NCLS], f32, isOutput=False)
    b1 = nc.declare_dram_parameter("b1", [F, 1], f32, isOutput=False)
    b2 = nc.declare_dram_parameter("b2", [F, 1], f32, isOutput=False)
    bc = nc.declare_dram_parameter("bc", [P, NCLS], f32, isOutput=False)
    # per-node row scales, laid out [P, NB] (column c = node block c)
    souts = nc.declare_dram_parameter("souts", [P, NB], f32, isOutput=False)
    sins = nc.declare_dram_parameter("sins", [P, NB], f32, isOutput=False)
    idx16 = nc.declare_dram_parameter("idx16", [NG, P, Rmax * 8], i16,
                                      isOutput=False)
    locm = nc.declare_dram_parameter("locm", [NG, P, Rmax], bf16,
                                     isOutput=False)
    iota_w = nc.declare_dram_parameter("iota_w", [P, capmax * P], bf16,
                                       isOutput=False)
    ident = nc.declare_dram_parameter("ident", [P, P], f32, isOutput=False)
    logits = nc.declare_dram_parameter("logits", [NPC, NCLS], f32,
                                       isOutput=True)

    with tile.TileContext(nc) as tc:
        with (
            tc.tile_pool(name="dram", bufs=1, space="DRAM") as dram,
            tc.tile_pool(name="consts", bufs=1) as consts,
            tc.tile_pool(name="hT", bufs=1) as hTp,
            tc.tile_pool(name="lhs", bufs=3) as lhsp,
            tc.tile_pool(name="gst", bufs=3) as gstp,
            tc.tile_pool(name="meta", bufs=3) as metap,
            tc.tile_pool(name="gat", bufs=3) as gatp,
            tc.tile_pool(name="oh", bufs=4) as ohp,
            tc.tile_pool(name="t1", bufs=3) as t1p,
            tc.tile_pool(name="hsl", bufs=3) as hslp,
            tc.tile_pool(name="out", bufs=3) as outp,
            tc.tile_pool(name="psA", bufs=2, space="PSUM") as psA,
            tc.tile_pool(name="psB", bufs=3, space="PSUM") as psB,
            tc.tile_pool(name="psT", bufs=2, space="PSUM") as psT,
            tc.tile_pool(name="psC", bufs=1, space="PSUM") as psC,
        ):
            # message tables: bf16 pair (hi | lo) per node row, 512B rows
            g_loc = dram.tile([NPC, 2 * F], bf16, name="g_loc")
            g1_full = dram.tile([NPAD, 2 * F], bf16, addr_space="Shared",
                                name="g1_full")
            g2_full = dram.tile([NPAD, 2 * F], bf16, addr_space="Shared",
                                name="g2_full")

            W1_sb = consts.tile([F, F], f32, name="W1_sb")
            nc.sync.dma_start(W1_sb[:], W1[:])
            W2_sb = consts.tile([F, F], f32, name="W2_sb")
            nc.sync.dma_start(W2_sb[:], W2[:])
            Wc_sb = consts.tile([F, NCLS], f32, name="Wc_sb")
            nc.sync.dma_start(Wc_sb[:], Wc[:])
            b1_sb = consts.tile([F, 1], f32, name="b1_sb")
            nc.sync.dma_start(b1_sb[:], b1[:])
            b2_sb = consts.tile([F, 1], f32, name="b2_sb")
            nc.sync.dma_start(b2_sb[:], b2[:])
            bc_sb = consts.tile([P, NCLS], f32, name="bc_sb")
            nc.sync.dma_start(bc_sb[:], bc[:])
            iota_sb = consts.tile([P, capmax * P], bf16, name="iota_sb")
            nc.sync.dma_start(iota_sb[:], iota_w[:])
            ident_sb = consts.tile([P, P], f32, name="ident_sb")
            nc.sync.dma_start(ident_sb[:], ident[:])
            souts_sb = consts.tile([P, NB], f32, name="souts_sb")
            nc.sync.dma_start(souts_sb[:], souts[:])
            sins_sb = consts.tile([P, NB], f32, name="sins_sb")
            nc.sync.dma_start(sins_sb[:], sins[:])

            # h1T split into per-group tiles so layer-2 stage A chunks only
            # depend on their own group's stage-B1 output
            GQ = cfg.group
            h1T = [hTp.tile([F, len(geom.grp_blocks[g]) * P], f32,
                            name=f"h1T_{g}", tag=f"hT{g}")
                   for g in range(NG)]

            def h1T_slice(c):
                g, i = c // GQ, c % GQ
                return h1T[g][:, i * P:(i + 1) * P]

            def stage_a(layer, W_sb, g_dst, chunks=None):
                for c in (range(NB) if chunks is None else chunks):
                    if layer == 1:
                        lhsT = lhsp.tile([F, P], f32, name="lhsT", tag="lhsT")
                        nc.scalar.dma_start(lhsT[:], xT[:, c * P:(c + 1) * P])
                        lhs_ap = lhsT[:]
                    else:
                        lhs_ap = h1T_slice(c)
                    pa = psA.tile([P, F], f32, name="pa", tag="pa")
                    nc.tensor.matmul(pa[:], lhs_ap, W_sb[:], start=True,
                                     stop=True)
                    gst = gstp.tile([P, F], f32, name="gst", tag="gst")
                    nc.scalar.activation(
                        out=gst[:], in_=pa[:],
                        func=mybir.ActivationFunctionType.Copy,
                        scale=souts_sb[:, c:c + 1])
                    # split into bf16 (hi | lo) pair: hi = bf16(g),
                    # lo = bf16(g - hi) -> hi + lo ~ g to ~2^-16 rel
                    g2t = gstp.tile([P, 2 * F], bf16, name="g2t", tag="g2t")
                    nc.vector.tensor_copy(out=g2t[:, :F], in_=gst[:])
                    nc.vector.tensor_tensor(
                        out=g2t[:, F:], in0=gst[:], in1=g2t[:, :F],
                        op=mybir.AluOpType.subtract)
                    nc.scalar.dma_start(g_dst[c * P:(c + 1) * P, :], g2t[:])

            def stage_b(layer, g_full, b_sb, hT_out, post_group=None):
                for g in range(NG):
                    Rg = geom.Rg[g]
                    idx = metap.tile([P, Rmax * 8], i16, name="idx",
                                     tag="idx")
                    nc.sync.dma_start(idx[:, :Rg * 8], idx16[g, :, :Rg * 8])
                    loc = metap.tile([P, Rmax], bf16, name="loc", tag="loc")
                    nc.sync.dma_start(loc[:, :Rg], locm[g, :, :Rg])
                    gat = gatp.tile([P, Rmax * 2 * F], bf16, name="gat",
                                    tag="gat")
                    MAXSUB = 64  # calls capped by packing below
                    for c in range(NCH):
                        c0, c1 = geom.chunk_rng[g][c]
                        for s0 in range(c0, c1, MAXSUB):
                            s1 = min(s0 + MAXSUB, c1)
                            n_idx = (s1 - s0) * P
                            out_ap = gat[:, s0 * 2 * F:s1 * 2 * F].rearrange(
                                "p (s f) -> p s f", s=s1 - s0)
                            nc.gpsimd.dma_gather(
                                out_ap=out_ap,
                                in_ap=g_full[c * CH:min((c + 1) * CH, NPAD), :],
                                idxs_ap=idx[:, s0 * 8:s1 * 8],
                                num_idxs=n_idx,
                                num_idxs_reg=n_idx,
                                elem_size=2 * F,
                                single_packet=False,
                            )
                    for i, b in enumerate(geom.grp_blocks[g]):
                        segs = geom.blk_segs[b]
                        pb = psB.tile([P, F], f32, name="pb", tag="pb")
                        n_segs = len(segs)
                        ti = 0
                        n_sub = sum(n for _, n in segs)
                        for s0, n in segs:
                            oh = ohp.tile([P, capmax * P], bf16, name="oh",
                                          tag="oh")
                            nc.vector.tensor_tensor(
                                out=oh[:, :n * P].rearrange(
                                    "p (s f) -> p s f", s=n),
                                in0=iota_sb[:, :n * P].rearrange(
                                    "p (s f) -> p s f", s=n),
                                in1=loc[:, s0:s0 + n].to_broadcast([P, n, P]),
                                op=mybir.AluOpType.is_equal)
                            for k in range(n):
                                t = s0 + k
                                nc.tensor.matmul(
                                    pb[:], oh[:, k * P:(k + 1) * P],
                                    gat[:, t * 2 * F:t * 2 * F + F],
                                    start=(ti == 0), stop=False)
                                nc.tensor.matmul(
                                    pb[:], oh[:, k * P:(k + 1) * P],
                                    gat[:, t * 2 * F + F:(t + 1) * 2 * F],
                                    start=False, stop=(ti == n_sub - 1))
                                ti += 1
                        t1 = t1p.tile([P, F], f32, name="t1", tag="t1")
                        nc.scalar.activation(
                            out=t1[:], in_=pb[:],
                            func=mybir.ActivationFunctionType.Copy,
                            scale=sins_sb[:, b:b + 1])
                        pt = psT.tile([F, P], f32, name="pt", tag="pt")
                        nc.tensor.transpose(pt[:], t1[:], ident_sb[:])
                        if layer == 1:
                            nc.scalar.activation(
                                out=h1T[g][:, i * P:(i + 1) * P], in_=pt[:],
                                func=mybir.ActivationFunctionType.Relu,
                                bias=b_sb[:, :1])
                        else:
                            hsl = hslp.tile([F, P], f32, name="hsl",
                                            tag="hsl")
                            nc.scalar.activation(
                                out=hsl[:], in_=pt[:],
                                func=mybir.ActivationFunctionType.Relu,
                                bias=b_sb[:, :1])
                            pc = psC.tile([P, NCLS], f32, name="pc",
                                          tag="pc")
                            nc.tensor.matmul(pc[:], hsl[:], Wc_sb[:],
                                             start=True, stop=True)
                            o = outp.tile([P, NCLS], f32, name="o", tag="o")
                            nc.vector.tensor_tensor(
                                out=o[:], in0=pc[:], in1=bc_sb[:],
                                op=mybir.AluOpType.add)
                            nc.sync.dma_start(
                                logits[b * P:(b + 1) * P, :], o[:])
                    if post_group is not None:
                        post_group(g)

            def all_gather(g_full):
                if single_core_sim:
                    nc.sync.dma_start(g_full[:NPC, :], g_loc[:])
                else:
                    nc.gpsimd.collective_compute(
                        "AllGather", mybir.AluOpType.bypass,
                        replica_groups=[list(range(cfg.n_cores))],
                        ins=[g_loc[:]], outs=[g_full[:]])

            stage_a(1, W1_sb, g_loc)
            all_gather(g1_full)
            stage_b(1, g1_full, b1_sb, h1T)
            stage_a(2, W2_sb, g_loc)
            all_gather(g2_full)
            stage_b(2, g2_full, b2_sb, None)

    nc.compile()
    return nc


def run(cfg: Cfg, features, src, dst, W1, b1, W2, b2, Wc, bc,
        trace=False, return_results=False):
    F, NPC, NPAD = cfg.in_feats, cfg.npc, cfg.npad
    n = cfg.n_nodes
    src = np.asarray(src).astype(np.int64)
    dst = np.asarray(dst).astype(np.int64)
    features = np.asarray(features, np.float32)
    deg_out = np.bincount(src, minlength=NPAD).astype(np.float32)
    deg_in = np.bincount(dst, minlength=NPAD).astype(np.float32)
    s_out_old = 1.0 / np.sqrt(np.maximum(deg_out, 1.0))
    s_in_old = 1.0 / np.sqrt(np.maximum(deg_in, 1.0))

    geom, node_new, idx16, loc = preprocess(cfg, src, dst)

    x_new = np.zeros((NPAD, F), np.float32)
    x_new[node_new[:n]] = features
    s_out = np.ones(NPAD, np.float32)
    s_out[node_new] = s_out_old
    s_in = np.ones(NPAD, np.float32)
    s_in[node_new] = s_in_old
    xT_full = np.ascontiguousarray(x_new.T)

    iota_np = np.tile(np.arange(P, dtype=np_bf16), (P, geom.capmax))
    ident_np = np.eye(P, dtype=np.float32)
    bc_b = np.tile(np.asarray(bc, np.float32)[None, :], (P, 1))

    in_maps = []
    for m in range(cfg.n_cores):
        sl = slice(m * NPC, (m + 1) * NPC)
        in_maps.append({
            "xT": np.ascontiguousarray(xT_full[:, sl]),
            "W1": np.asarray(W1, np.float32),
            "W2": np.asarray(W2, np.float32),
            "Wc": np.asarray(Wc, np.float32),
            "b1": np.asarray(b1, np.float32)[:, None],
            "b2": np.asarray(b2, np.float32)[:, None],
            "bc": bc_b,
            "souts": np.ascontiguousarray(
                s_out[sl].reshape(cfg.nb, P).T),
            "sins": np.ascontiguousarray(
                s_in[sl].reshape(cfg.nb, P).T),
            "idx16": idx16[m],
            "locm": loc[m],
            "iota_w": iota_np,
            "ident": ident_np,
        })

    nc = build_program(cfg, geom)
    last_err = None
    for _attempt in range(3):
        try:
            res = run_bass_kernel_spmd(nc, in_maps, list(range(cfg.n_cores)),
                                       trace=trace)
            break
        except Exception as e:  # transient axon worker hiccups
            last_err = e
    else:
        raise last_err
    out_new = np.concatenate([r["logits"] for r in res.results], axis=0)
    out = out_new[node_new[:n]].astype(np.float32)
    if return_results:
        return out, res
    return out


def kernel(features, src, dst, W1, b1, W2, b2, Wc, bc):
    return run(CFG, features, src, dst, W1, b1, W2, b2, Wc, bc)



# revision 2
# speedup vs baseline: 1.0170x; 1.0170x over previous
"""Trainium2 Bass kernel for a 2-layer GCN (DGL GraphConv, norm='both').

Reference computation (per layer):
    h = relu( deg_in^-0.5 * segment_sum( ((x * deg_out^-0.5) @ W)[src], dst ) + b )
then logits = h2 @ Wc + bc.

Distribution: nodes are relabeled into 128-wide blocks, blocks are
load-balanced across the 8 NeuronCores (snake assignment by edge count),
giving every core an equal, structurally identical workload (SPMD: one
program, per-core data). Per layer:
  stage A: each core computes g = (x @ W) * s_out for its node shard
           (bf16 message table, 256B rows)
  AllGather: g shards -> full g table in every core's DRAM
  stage B: blocks are processed in groups of 7; slots are pooled per
    (group, chunk) cell with exact per-block capacities (no 128-rounding
    of block intervals), one dma_gather per cell; the per-block
    segment-sum is one-hot x messages matmuls accumulated in PSUM, with
    block membership encoded in loc via a parity offset (adjacent blocks
    alternate +0/+128) so boundary subtiles shared by two blocks stay
    exact; epilogue scales by s_in, transposes, adds bias, relu.
Layer 2's epilogue is fused with the classifier: logits = h2 @ Wc + bc,
written per shard; the host reassembles and inverse-permutes.

All index preprocessing (degree counts, edge sorting/padding, relabeling)
is host-side numpy on integer graph structure; float math is on device.
"""
import math
from dataclasses import dataclass

import numpy as np

import concourse.bacc as bacc
import concourse.mybir as mybir
import concourse.tile as tile
from concourse.ap import AP
from concourse.bass_utils import run_bass_kernel_spmd

f32 = mybir.dt.float32
bf16 = mybir.dt.bfloat16
i16 = mybir.dt.int16

P = 128  # partitions / node block size

# numpy view of bfloat16 for host-side constant/input arrays
import ml_dtypes  # noqa: E402  (ships with jax)

np_bf16 = ml_dtypes.bfloat16


@dataclass
class Cfg:
    n_nodes: int = 100000
    in_feats: int = 128
    num_classes: int = 4
    n_cores: int = 8
    nb: int = 98          # node blocks per core
    chunk: int = 25088    # gather sub-table rows (int16-addressable)
    group: int = 7        # blocks per gather group

    @property
    def npc(self):        # nodes per core
        return self.nb * P

    @property
    def npad(self):       # padded node count
        return self.n_cores * self.npc

    @property
    def n_chunks(self):
        return math.ceil(self.npad / self.chunk)

    @property
    def n_groups(self):
        return math.ceil(self.nb / self.group)


CFG = Cfg()

MAXSUB = 60  # max subtiles per dma_gather call


class Geometry:
    """Static slot layout: per (group, chunk) cell, blocks get exact
    (cross-core max) capacities packed back-to-back; cells are
    subtile-aligned. Identical across cores."""

    def __init__(self, cfg: Cfg, bcap: np.ndarray):
        NG, NCH, G = cfg.n_groups, cfg.n_chunks, cfg.group
        self.bcap = bcap                      # [NG, NCH, G]
        self.cellcap = bcap.sum(axis=2)       # [NG, NCH]
        self.S = -(-self.cellcap // P)        # subtiles per cell
        # cell subtile base within group (cumulative over chunks)
        self.sbase = np.zeros((NG, NCH), np.int64)
        self.sbase[:, 1:] = np.cumsum(self.S, axis=1)[:, :-1]
        self.Bbase = self.sbase * P           # cell slot base
        self.Rg = self.S.sum(axis=1)          # [NG] subtiles per group
        self.Rmax = int(self.Rg.max())
        # block slot intervals within cell
        self.off = np.zeros((NG, NCH, G), np.int64)
        self.off[:, :, 1:] = np.cumsum(bcap, axis=2)[:, :, :-1]
        a = self.Bbase[:, :, None] + self.off           # incl. group base
        e = a + bcap
        self.w0 = a // P                      # window subtile start
        self.w1 = -(-e // P)                  # window subtile end (excl)
        self.w1 = np.where(bcap > 0, self.w1, self.w0)
        self.WMAX = int((self.w1 - self.w0).max())


def preprocess(cfg: Cfg, src: np.ndarray, dst: np.ndarray):
    """Relabel nodes, sort edges into group-pooled gather metadata.

    Returns (geom, node_new, idx16, locd):
      idx16[m]: [NG, P, Rmax*8] int16 indices (16-wrapped, 8x repl)
      locd[m]:  [NG, P, Rmax*2] bf16 parity-encoded local dst
                (pairwise-duplicated for the packed-last-dim DVE mode),
                1000 for pads
    Group slot j -> partition j%128, group subtile j//128.
    """
    ncores, nb, nch, CH = cfg.n_cores, cfg.nb, cfg.n_chunks, cfg.chunk
    G, NG = cfg.group, cfg.n_groups
    n_blocks = ncores * nb

    # block load balancing: snake-assign blocks by edge count
    blk_tot = np.bincount(dst >> 7, minlength=n_blocks)
    order = np.argsort(-blk_tot, kind="stable")
    rank = np.arange(n_blocks)
    lane = rank % ncores
    rev = (rank // ncores) % 2 == 1
    core_of_rank = np.where(rev, ncores - 1 - lane, lane)
    core_of_old = np.empty(n_blocks, np.int64)
    pos_of_old = np.empty(n_blocks, np.int64)
    core_of_old[order] = core_of_rank
    pos_of_old[order] = rank // ncores
    new_blk_of_old = core_of_old * nb + pos_of_old
    node_ar = np.arange(cfg.npad, dtype=np.int64)
    node_new = new_blk_of_old[node_ar >> 7] * P + (node_ar & 127)

    src_n = node_new[src]
    dst_n = node_new[dst]

    blk = dst_n >> 7
    m_arr = blk // nb
    b_arr = blk % nb
    g_arr = b_arr // G
    i_arr = b_arr % G
    c_arr = src_n // CH

    key = ((m_arr * NG + g_arr) * nch + c_arr) * G + i_arr
    counts = np.bincount(key, minlength=ncores * NG * nch * G).reshape(
        ncores, NG, nch, G)
    # exact per-block capacity = cross-core max; floor 128 so a subtile
    # never spans more than two (parity-distinct) block intervals
    bcap = np.maximum(counts.max(axis=0), P)
    geom = Geometry(cfg, bcap)

    # per-edge slot within its group's slot space
    perm = np.lexsort((src_n, i_arr, c_arr, g_arr, m_arr))
    src_s = src_n[perm]
    m_s = m_arr[perm]
    g_s = g_arr[perm]
    c_s = c_arr[perm]
    i_s = i_arr[perm]
    key_s = key[perm]
    ecum = np.zeros(ncores * NG * nch * G + 1, np.int64)
    np.cumsum(counts.reshape(-1), out=ecum[1:])
    within = np.arange(len(src_s)) - ecum[key_s]
    slot = geom.Bbase[g_s, c_s] + geom.off[g_s, c_s, i_s] + within

    Rmax = geom.Rmax
    locv = ((dst_n[perm] & 127) + P * (i_s & 1)).astype(np.float32)
    loc = np.full((ncores, NG, P, Rmax), 1000.0, np_bf16)
    t_arr = slot // P
    p_arr = slot % P
    loc[m_s, g_s, p_arr, t_arr] = locv.astype(np_bf16)
    locd = np.repeat(loc, 2, axis=3)  # pairwise duplicate along slots

    val = (src_s - c_s * CH).astype(np.int16)
    c16 = slot // 16
    r16 = slot % 16
    flat = np.zeros((ncores, NG, 16, Rmax * 8), np.int16)
    flat[m_s, g_s, r16, c16] = val
    idx16 = np.tile(flat, (1, 1, 8, 1))
    return geom, node_new, idx16, locd


def build_program(cfg: Cfg, geom: Geometry, single_core_sim=False):
    F = cfg.in_feats
    NB, NPC, NPAD = cfg.nb, cfg.npc, cfg.npad
    NCH, CH, NG, G = cfg.n_chunks, cfg.chunk, cfg.n_groups, cfg.group
    NCLS = cfg.num_classes
    Rmax, WMAX = geom.Rmax, geom.WMAX

    n_dev = 1 if single_core_sim else cfg.n_cores
    nc = bacc.Bacc("TRN2", target_bir_lowering=False, debug=False,
                   num_devices=n_dev)

    xT = nc.declare_dram_parameter("xT", [F, NPC], bf16, isOutput=False)
    W1 = nc.declare_dram_parameter("W1", [F, F], bf16, isOutput=False)
    W2 = nc.declare_dram_parameter("W2", [F, F], bf16, isOutput=False)
    Wc = nc.declare_dram_parameter("Wc", [F, NCLS], bf16, isOutput=False)
    b1 = nc.declare_dram_parameter("b1", [F, 1], f32, isOutput=False)
    b2 = nc.declare_dram_parameter("b2", [F, 1], f32, isOutput=False)
    bc = nc.declare_dram_parameter("bc", [P, NCLS], f32, isOutput=False)
    souts = nc.declare_dram_parameter("souts", [P, NB], f32, isOutput=False)
    sins = nc.declare_dram_parameter("sins", [P, NB], f32, isOutput=False)
    idx16 = nc.declare_dram_parameter("idx16", [NG, P, Rmax * 8], i16,
                                      isOutput=False)
    locm = nc.declare_dram_parameter("locm", [NG, P, Rmax * 2], bf16,
                                     isOutput=False)
    iota_w = nc.declare_dram_parameter("iota_w", [P, 2 * WMAX * P], bf16,
                                       isOutput=False)
    ident = nc.declare_dram_parameter("ident", [P, P], f32, isOutput=False)
    logits = nc.declare_dram_parameter("logits", [NPC, NCLS], f32,
                                       isOutput=True)

    with tile.TileContext(nc) as tc:
        with (
            tc.tile_pool(name="dram", bufs=1, space="DRAM") as dram,
            tc.tile_pool(name="consts", bufs=1) as consts,
            tc.tile_pool(name="hT", bufs=1) as hTp,
            tc.tile_pool(name="gst", bufs=3) as gstp,
            tc.tile_pool(name="gat", bufs=2) as gatp,
            tc.tile_pool(name="oh", bufs=4) as ohp,
            tc.tile_pool(name="t1", bufs=3) as t1p,
            tc.tile_pool(name="hsl", bufs=3) as hslp,
            tc.tile_pool(name="out", bufs=3) as outp,
            tc.tile_pool(name="psA", bufs=2, space="PSUM") as psA,
            tc.tile_pool(name="psB", bufs=3, space="PSUM") as psB,
            tc.tile_pool(name="psT", bufs=2, space="PSUM") as psT,
            tc.tile_pool(name="psC", bufs=1, space="PSUM") as psC,
        ):
            # message tables: single bf16 per node row, 256B rows
            g_loc = dram.tile([NPC, F], bf16, name="g_loc")
            g1_full = dram.tile([NPAD, F], bf16, addr_space="Shared",
                                name="g1_full")
            g2_full = dram.tile([NPAD, F], bf16, addr_space="Shared",
                                name="g2_full")

            W1_sb = consts.tile([F, F], bf16, name="W1_sb")
            nc.sync.dma_start(W1_sb[:], W1[:])
            W2_sb = consts.tile([F, F], bf16, name="W2_sb")
            nc.sync.dma_start(W2_sb[:], W2[:])
            Wc_sb = consts.tile([F, NCLS], bf16, name="Wc_sb")
            nc.sync.dma_start(Wc_sb[:], Wc[:])
            b1_sb = consts.tile([F, 1], f32, name="b1_sb")
            nc.sync.dma_start(b1_sb[:], b1[:])
            b2_sb = consts.tile([F, 1], f32, name="b2_sb")
            nc.sync.dma_start(b2_sb[:], b2[:])
            bc_sb = consts.tile([P, NCLS], f32, name="bc_sb")
            nc.sync.dma_start(bc_sb[:], bc[:])
            iota_sb = consts.tile([P, 2 * WMAX * P], bf16, name="iota_sb")
            nc.sync.dma_start(iota_sb[:], iota_w[:])
            ident_sb = consts.tile([P, P], f32, name="ident_sb")
            nc.sync.dma_start(ident_sb[:], ident[:])
            souts_sb = consts.tile([P, NB], f32, name="souts_sb")
            nc.sync.dma_start(souts_sb[:], souts[:])
            sins_sb = consts.tile([P, NB], f32, name="sins_sb")
            nc.sync.dma_start(sins_sb[:], sins[:])

            # SBUF-resident xT and per-group metadata (shared by both layers)
            xT_sb = consts.tile([F, NPC], bf16, name="xT_sb")
            nc.scalar.dma_start(xT_sb[:], xT[:])
            idx_sb = []
            locd_sb = []
            for g in range(NG):
                Rg = int(geom.Rg[g])
                it = consts.tile([P, Rmax * 8], i16, name=f"idx_sb{g}")
                nc.sync.dma_start(it[:, :Rg * 8], idx16[g, :, :Rg * 8])
                lt = consts.tile([P, Rmax * 2], bf16, name=f"locd_sb{g}")
                nc.sync.dma_start(lt[:, :Rg * 2], locm[g, :, :Rg * 2])
                idx_sb.append(it)
                locd_sb.append(lt)

            h1T = [hTp.tile([F, G * P], bf16, name=f"h1T_{g}", tag=f"hT{g}")
                   for g in range(NG)]

            def stage_a(layer, W_sb, g_dst, chunks):
                for c in chunks:
                    if layer == 1:
                        lhs_ap = xT_sb[:, c * P:(c + 1) * P]
                    else:
                        lhs_ap = h1T[c // G][:, (c % G) * P:(c % G + 1) * P]
                    pa = psA.tile([P, F], f32, name="pa", tag="pa")
                    nc.tensor.matmul(pa[:], lhs_ap, W_sb[:], start=True,
                                     stop=True)
                    gst = gstp.tile([P, F], bf16, name="gst", tag="gst")
                    nc.scalar.activation(
                        out=gst[:], in_=pa[:],
                        func=mybir.ActivationFunctionType.Copy,
                        scale=souts_sb[:, c:c + 1])
                    nc.scalar.dma_start(g_dst[c * P:(c + 1) * P, :], gst[:])

            def one_hot(oh_ap, loc_tile, a0, n, parity):
                """oh[:, :n*P] = is_equal(iota+128*parity, locd[a0:a0+n])
                with a packed-pairs 4D AP so DVE runs in 2x_1p mode."""
                out4 = oh_ap.rearrange("p (t a b) -> p t a b", a=64, b=2)
                in0 = iota_sb[:, parity * WMAX * P:
                              parity * WMAX * P + n * P].rearrange(
                    "p (t a b) -> p t a b", a=64, b=2)
                sl = loc_tile[:, 2 * a0:2 * (a0 + n)]
                in1 = AP(sl.tensor, sl.offset,
                         [list(sl.ap[0]), [2, n], [0, 64], [1, 2]])
                nc.vector.tensor_tensor(out=out4, in0=in0, in1=in1,
                                        op=mybir.AluOpType.is_equal)

            def stage_b(layer, g_full, b_sb, post_group):
                for g in range(NG):
                    gat = gatp.tile([P, Rmax * F], bf16, name="gat",
                                    tag="gat")
                    for c in range(NCH):
                        ncell = int(geom.cellcap[g, c])
                        if ncell == 0:
                            continue
                        S = int(geom.S[g, c])
                        sb = int(geom.sbase[g, c])   # cell subtile base
                        for s0 in range(0, S, MAXSUB):
                            s1 = min(s0 + MAXSUB, S)
                            n_idx = min(ncell - s0 * P, (s1 - s0) * P)
                            out_ap = gat[:, (sb + s0) * F:
                                         (sb + s1) * F].rearrange(
                                "p (s f) -> p s f", s=s1 - s0)
                            col0 = sb * 8 + s0 * 8
                            col1 = col0 + (-(-n_idx // 16))
                            nc.gpsimd.dma_gather(
                                out_ap=out_ap,
                                in_ap=g_full[c * CH:(c + 1) * CH, :],
                                idxs_ap=idx_sb[g][:, col0:col1],
                                num_idxs=n_idx,
                                num_idxs_reg=n_idx,
                                elem_size=F,
                                single_packet=False,
                            )
                    for i in range(G):
                        b = g * G + i
                        segs = [(int(geom.w0[g, c, i]), int(geom.w1[g, c, i]))
                                for c in range(NCH)
                                if geom.w1[g, c, i] > geom.w0[g, c, i]]
                        ntot = sum(a1 - a0 for a0, a1 in segs)
                        pb = psB.tile([P, F], f32, name="pb", tag="pb")
                        if ntot == 0:
                            nc.vector.memset(pb[:], 0.0)
                        ti = 0
                        for a0, a1 in segs:
                            n = a1 - a0
                            oh = ohp.tile([P, WMAX * P], bf16, name="oh",
                                          tag="oh")
                            one_hot(oh[:, :n * P], locd_sb[g], a0, n, i & 1)
                            for k in range(n):
                                nc.tensor.matmul(
                                    pb[:], oh[:, k * P:(k + 1) * P],
                                    gat[:, (a0 + k) * F:(a0 + k + 1) * F],
                                    start=(ti == 0), stop=(ti == ntot - 1))
                                ti += 1
                        t1 = t1p.tile([P, F], f32, name="t1", tag="t1")
                        nc.scalar.activation(
                            out=t1[:], in_=pb[:],
                            func=mybir.ActivationFunctionType.Copy,
                            scale=sins_sb[:, b:b + 1])
                        pt = psT.tile([F, P], f32, name="pt", tag="pt")
                        nc.tensor.transpose(pt[:], t1[:], ident_sb[:])
                        if layer == 1:
                            nc.scalar.activation(
                                out=h1T[g][:, i * P:(i + 1) * P], in_=pt[:],
                                func=mybir.ActivationFunctionType.Relu,
                                bias=b_sb[:, :1])
                        else:
                            hsl = hslp.tile([F, P], bf16, name="hsl",
                                            tag="hsl")
                            nc.scalar.activation(
                                out=hsl[:], in_=pt[:],
                                func=mybir.ActivationFunctionType.Relu,
                                bias=b_sb[:, :1])
                            pc = psC.tile([P, NCLS], f32, name="pc",
                                          tag="pc")
                            nc.tensor.matmul(pc[:], hsl[:], Wc_sb[:],
                                             start=True, stop=True)
                            o = outp.tile([P, NCLS], f32, name="o", tag="o")
                            nc.vector.tensor_tensor(
                                out=o[:], in0=pc[:], in1=bc_sb[:],
                                op=mybir.AluOpType.add)
                            nc.sync.dma_start(
                                logits[b * P:(b + 1) * P, :], o[:])
                    if post_group is not None:
                        post_group(g)

            def all_gather(g_full):
                if single_core_sim:
                    nc.sync.dma_start(g_full[:NPC, :], g_loc[:])
                else:
                    nc.gpsimd.collective_compute(
                        "AllGather", mybir.AluOpType.bypass,
                        replica_groups=[list(range(cfg.n_cores))],
                        ins=[g_loc[:]], outs=[g_full[:]])

            stage_a(1, W1_sb, g_loc, range(NB))
            all_gather(g1_full)
            stage_b(1, g1_full, b1_sb,
                    post_group=lambda g: stage_a(
                        2, W2_sb, g_loc, range(g * G, (g + 1) * G)))
            all_gather(g2_full)
            stage_b(2, g2_full, b2_sb, None)

    nc.compile()
    return nc


def run(cfg: Cfg, features, src, dst, W1, b1, W2, b2, Wc, bc,
        trace=False, return_results=False):
    F, NPC, NPAD = cfg.in_feats, cfg.npc, cfg.npad
    n = cfg.n_nodes
    src = np.asarray(src).astype(np.int64)
    dst = np.asarray(dst).astype(np.int64)
    features = np.asarray(features, np.float32)
    deg_out = np.bincount(src, minlength=NPAD).astype(np.float32)
    deg_in = np.bincount(dst, minlength=NPAD).astype(np.float32)
    s_out_old = 1.0 / np.sqrt(np.maximum(deg_out, 1.0))
    s_in_old = 1.0 / np.sqrt(np.maximum(deg_in, 1.0))

    geom, node_new, idx16, locd = preprocess(cfg, src, dst)

    x_new = np.zeros((NPAD, F), np.float32)
    x_new[node_new[:n]] = features
    s_out = np.ones(NPAD, np.float32)
    s_out[node_new] = s_out_old
    s_in = np.ones(NPAD, np.float32)
    s_in[node_new] = s_in_old
    xT_full = np.ascontiguousarray(x_new.T).astype(np_bf16)

    iota_pat = np.tile(np.arange(P, dtype=np.float32), geom.WMAX)
    iota_np = np.concatenate([iota_pat, iota_pat + P]).astype(np_bf16)
    iota_np = np.tile(iota_np, (P, 1))
    ident_np = np.eye(P, dtype=np.float32)
    bc_b = np.tile(np.asarray(bc, np.float32)[None, :], (P, 1))

    in_maps = []
    for m in range(cfg.n_cores):
        sl = slice(m * NPC, (m + 1) * NPC)
        in_maps.append({
            "xT": np.ascontiguousarray(xT_full[:, sl]),
            "W1": np.asarray(W1, np.float32).astype(np_bf16),
            "W2": np.asarray(W2, np.float32).astype(np_bf16),
            "Wc": np.asarray(Wc, np.float32).astype(np_bf16),
            "b1": np.asarray(b1, np.float32)[:, None],
            "b2": np.asarray(b2, np.float32)[:, None],
            "bc": bc_b,
            "souts": np.ascontiguousarray(
                s_out[sl].reshape(cfg.nb, P).T),
            "sins": np.ascontiguousarray(
                s_in[sl].reshape(cfg.nb, P).T),
            "idx16": idx16[m],
            "locm": locd[m],
            "iota_w": iota_np,
            "ident": ident_np,
        })

    nc = build_program(cfg, geom)
    last_err = None
    for _attempt in range(3):
        try:
            res = run_bass_kernel_spmd(nc, in_maps, list(range(cfg.n_cores)),
                                       trace=trace)
            break
        except Exception as e:  # transient axon worker hiccups
            last_err = e
    else:
        raise last_err
    out_new = np.concatenate([r["logits"] for r in res.results], axis=0)
    out = out_new[node_new[:n]].astype(np.float32)
    if return_results:
        return out, res
    return out


def kernel(features, src, dst, W1, b1, W2, b2, Wc, bc):
    return run(CFG, features, src, dst, W1, b1, W2, b2, Wc, bc)


# revision 13
# speedup vs baseline: 1.0771x; 1.0590x over previous
"""Trainium2 Bass kernel for a 2-layer GCN (DGL GraphConv, norm='both').

Reference computation (per layer):
    h = relu( deg_in^-0.5 * segment_sum( ((x * deg_out^-0.5) @ W)[src], dst ) + b )
then logits = h2 @ Wc + bc.

Distribution: nodes are relabeled into 128-wide blocks, blocks are
load-balanced across the 8 NeuronCores (snake assignment by edge count),
giving every core an equal, structurally identical workload (SPMD: one
program, per-core data). Per layer:
  stage A: each core computes g = (x @ W) * s_out for its node shard
           (bf16 message table, 256B rows)
  AllGather: g shards -> full g table in every core's DRAM
  stage B: blocks are processed in groups of 7; slots are pooled per
    (group, chunk) cell with exact per-block capacities (no 128-rounding
    of block intervals), one dma_gather per cell; the per-block
    segment-sum is messages^T x one-hot matmuls accumulated in PSUM
    (messages stationary, so the result lands pre-transposed [F, P]),
    with block membership encoded in loc via a parity offset (adjacent
    blocks alternate +0/+128) so boundary subtiles shared by two blocks
    stay exact; epilogue is a single relu (b1 = b2 = 0, so
    relu(s_in * agg) = s_in * relu(agg) and the s_in scale folds into
    the NEXT stage-A per-node scale: souts2 = s_out * s_in).
Layer 2's epilogue is fused with the classifier:
    logits = (relu(agg2) @ Wc) * s_in2 + bc,
written per shard; the host reassembles and inverse-permutes.

All index preprocessing (degree counts, edge sorting/padding, relabeling)
is host-side numpy on integer graph structure; float math is on device.
"""
import math
from dataclasses import dataclass

import numpy as np

import concourse.bacc as bacc
import concourse.mybir as mybir
import concourse.tile as tile
from concourse.ap import AP
from concourse.bass_utils import run_bass_kernel_spmd

f32 = mybir.dt.float32
bf16 = mybir.dt.bfloat16
i16 = mybir.dt.int16

P = 128  # partitions / node block size

# numpy view of bfloat16 for host-side constant/input arrays
import ml_dtypes  # noqa: E402  (ships with jax)

np_bf16 = ml_dtypes.bfloat16


@dataclass
class Cfg:
    n_nodes: int = 100000
    in_feats: int = 128
    num_classes: int = 4
    n_cores: int = 8
    nb: int = 98          # node blocks per core
    chunk: int = 25088    # gather sub-table rows (int16-addressable)
    group: int = 7        # blocks per gather group

    @property
    def npc(self):        # nodes per core
        return self.nb * P

    @property
    def npad(self):       # padded node count
        return self.n_cores * self.npc

    @property
    def n_chunks(self):
        return math.ceil(self.npad / self.chunk)

    @property
    def n_groups(self):
        return math.ceil(self.nb / self.group)


CFG = Cfg()

MAXSUB = 60  # max subtiles per dma_gather call


class Geometry:
    """Static slot layout: per (group, chunk) cell, blocks get exact
    (cross-core max) capacities packed back-to-back; cells are
    subtile-aligned. Identical across cores."""

    def __init__(self, cfg: Cfg, bcap: np.ndarray):
        NG, NCH, G = cfg.n_groups, cfg.n_chunks, cfg.group
        self.bcap = bcap                      # [NG, NCH, G]
        self.cellcap = bcap.sum(axis=2)       # [NG, NCH]
        self.S = -(-self.cellcap // P)        # subtiles per cell
        # cell subtile base within group (cumulative over chunks)
        self.sbase = np.zeros((NG, NCH), np.int64)
        self.sbase[:, 1:] = np.cumsum(self.S, axis=1)[:, :-1]
        self.Bbase = self.sbase * P           # cell slot base
        self.Rg = self.S.sum(axis=1)          # [NG] subtiles per group
        self.Rmax = int(self.Rg.max())
        # block slot intervals within cell
        self.off = np.zeros((NG, NCH, G), np.int64)
        self.off[:, :, 1:] = np.cumsum(bcap, axis=2)[:, :, :-1]
        a = self.Bbase[:, :, None] + self.off           # incl. group base
        e = a + bcap
        self.w0 = a // P                      # window subtile start
        self.w1 = -(-e // P)                  # window subtile end (excl)
        self.w1 = np.where(bcap > 0, self.w1, self.w0)
        self.WMAX = int((self.w1 - self.w0).max())


def preprocess(cfg: Cfg, src: np.ndarray, dst: np.ndarray):
    """Relabel nodes, sort edges into group-pooled gather metadata.

    Returns (geom, node_new, idx16, locd):
      idx16[m]: [NG, P, Rmax*8] int16 indices (16-wrapped, 8x repl)
      locd[m]:  [NG, P, Rmax*2] bf16 parity-encoded local dst
                (pairwise-duplicated for the packed-last-dim DVE mode),
                1000 for pads
    Group slot j -> partition j%128, group subtile j//128.
    """
    ncores, nb, nch, CH = cfg.n_cores, cfg.nb, cfg.n_chunks, cfg.chunk
    G, NG = cfg.group, cfg.n_groups
    n_blocks = ncores * nb

    # block load balancing: snake-assign blocks by edge count
    blk_tot = np.bincount(dst >> 7, minlength=n_blocks)
    order = np.argsort(-blk_tot, kind="stable")
    rank = np.arange(n_blocks)
    lane = rank % ncores
    rev = (rank // ncores) % 2 == 1
    core_of_rank = np.where(rev, ncores - 1 - lane, lane)
    core_of_old = np.empty(n_blocks, np.int64)
    pos_of_old = np.empty(n_blocks, np.int64)
    core_of_old[order] = core_of_rank
    pos_of_old[order] = rank // ncores
    new_blk_of_old = core_of_old * nb + pos_of_old
    node_ar = np.arange(cfg.npad, dtype=np.int64)
    node_new = new_blk_of_old[node_ar >> 7] * P + (node_ar & 127)

    src_n = node_new[src]
    dst_n = node_new[dst]

    blk = dst_n >> 7
    m_arr = blk // nb
    b_arr = blk % nb
    g_arr = b_arr // G
    i_arr = b_arr % G
    c_arr = src_n // CH

    key = ((m_arr * NG + g_arr) * nch + c_arr) * G + i_arr
    counts = np.bincount(key, minlength=ncores * NG * nch * G).reshape(
        ncores, NG, nch, G)
    # exact per-block capacity = cross-core max; floor 128 so a subtile
    # never spans more than two (parity-distinct) block intervals
    bcap = np.maximum(counts.max(axis=0), P)
    geom = Geometry(cfg, bcap)

    # per-edge slot within its group's slot space
    perm = np.lexsort((src_n, i_arr, c_arr, g_arr, m_arr))
    src_s = src_n[perm]
    m_s = m_arr[perm]
    g_s = g_arr[perm]
    c_s = c_arr[perm]
    i_s = i_arr[perm]
    key_s = key[perm]
    ecum = np.zeros(ncores * NG * nch * G + 1, np.int64)
    np.cumsum(counts.reshape(-1), out=ecum[1:])
    within = np.arange(len(src_s)) - ecum[key_s]
    slot = geom.Bbase[g_s, c_s] + geom.off[g_s, c_s, i_s] + within

    Rmax = geom.Rmax
    locv = ((dst_n[perm] & 127) + P * (i_s & 1)).astype(np.float32)
    loc = np.full((ncores, NG, P, Rmax), 1000.0, np_bf16)
    t_arr = slot // P
    p_arr = slot % P
    loc[m_s, g_s, p_arr, t_arr] = locv.astype(np_bf16)
    locd = np.repeat(loc, 2, axis=3)  # pairwise duplicate along slots

    val = (src_s - c_s * CH).astype(np.int16)
    c16 = slot // 16
    r16 = slot % 16
    flat = np.zeros((ncores, NG, 16, Rmax * 8), np.int16)
    flat[m_s, g_s, r16, c16] = val
    idx16 = np.tile(flat, (1, 1, 8, 1))
    return geom, node_new, idx16, locd


def build_program(cfg: Cfg, geom: Geometry, single_core_sim=False):
    F = cfg.in_feats
    NB, NPC, NPAD = cfg.nb, cfg.npc, cfg.npad
    NCH, CH, NG, G = cfg.n_chunks, cfg.chunk, cfg.n_groups, cfg.group
    NCLS = cfg.num_classes
    Rmax, WMAX = geom.Rmax, geom.WMAX

    n_dev = 1 if single_core_sim else cfg.n_cores
    nc = bacc.Bacc("TRN2", target_bir_lowering=False, debug=False,
                   num_devices=n_dev)

    xT = nc.declare_dram_parameter("xT", [F, NPC], bf16, isOutput=False)
    W1 = nc.declare_dram_parameter("W1", [F, F], bf16, isOutput=False)
    W2 = nc.declare_dram_parameter("W2", [F, F], bf16, isOutput=False)
    Wc = nc.declare_dram_parameter("Wc", [F, NCLS], bf16, isOutput=False)
    bc = nc.declare_dram_parameter("bc", [P, NCLS], f32, isOutput=False)
    souts1 = nc.declare_dram_parameter("souts1", [P, NB], f32, isOutput=False)
    souts2 = nc.declare_dram_parameter("souts2", [P, NB], f32, isOutput=False)
    sins = nc.declare_dram_parameter("sins", [P, NB], f32, isOutput=False)
    idx16 = nc.declare_dram_parameter("idx16", [NG, P, Rmax * 8], i16,
                                      isOutput=False)
    locm = nc.declare_dram_parameter("locm", [NG, P, Rmax * 2], bf16,
                                     isOutput=False)
    iota_w = nc.declare_dram_parameter("iota_w", [P, 2 * WMAX * P], bf16,
                                       isOutput=False)
    logits = nc.declare_dram_parameter("logits", [NPC, NCLS], f32,
                                       isOutput=True)

    with tile.TileContext(nc) as tc:
        with (
            tc.tile_pool(name="dram", bufs=1, space="DRAM") as dram,
            tc.tile_pool(name="consts", bufs=1) as consts,
            tc.tile_pool(name="hT", bufs=1) as hTp,
            tc.tile_pool(name="gst", bufs=3) as gstp,
            tc.tile_pool(name="gat", bufs=2) as gatp,
            tc.tile_pool(name="oh", bufs=4) as ohp,
            tc.tile_pool(name="hsl", bufs=3) as hslp,
            tc.tile_pool(name="out", bufs=3) as outp,
            tc.tile_pool(name="psA", bufs=2, space="PSUM") as psA,
            tc.tile_pool(name="psB", bufs=5, space="PSUM") as psB,
            tc.tile_pool(name="psC", bufs=1, space="PSUM") as psC,
        ):
            # message tables: single bf16 per node row, 256B rows
            g_loc = dram.tile([NPC, F], bf16, name="g_loc")
            g1_full = dram.tile([NPAD, F], bf16, addr_space="Shared",
                                name="g1_full")
            g2_full = dram.tile([NPAD, F], bf16, addr_space="Shared",
                                name="g2_full")

            W1_sb = consts.tile([F, F], bf16, name="W1_sb")
            nc.sync.dma_start(W1_sb[:], W1[:])
            W2_sb = consts.tile([F, F], bf16, name="W2_sb")
            nc.sync.dma_start(W2_sb[:], W2[:])
            Wc_sb = consts.tile([F, NCLS], bf16, name="Wc_sb")
            nc.sync.dma_start(Wc_sb[:], Wc[:])
            bc_sb = consts.tile([P, NCLS], f32, name="bc_sb")
            nc.sync.dma_start(bc_sb[:], bc[:])
            iota_sb = consts.tile([P, 2 * WMAX * P], bf16, name="iota_sb")
            nc.sync.dma_start(iota_sb[:], iota_w[:])
            souts1_sb = consts.tile([P, NB], f32, name="souts1_sb")
            nc.sync.dma_start(souts1_sb[:], souts1[:])
            souts2_sb = consts.tile([P, NB], f32, name="souts2_sb")
            nc.sync.dma_start(souts2_sb[:], souts2[:])
            sins_sb = consts.tile([P, NB], f32, name="sins_sb")
            nc.sync.dma_start(sins_sb[:], sins[:])

            # SBUF-resident xT and per-group metadata (shared by both layers)
            xT_sb = consts.tile([F, NPC], bf16, name="xT_sb")
            nc.sync.dma_start(xT_sb[:], xT[:])
            idx_sb = []
            locd_sb = []
            for g in range(NG):
                Rg = int(geom.Rg[g])
                it = consts.tile([P, Rmax * 8], i16, name=f"idx_sb{g}")
                nc.sync.dma_start(it[:, :Rg * 8], idx16[g, :, :Rg * 8])
                lt = consts.tile([P, Rmax * 2], bf16, name=f"locd_sb{g}")
                nc.sync.dma_start(lt[:, :Rg * 2], locm[g, :, :Rg * 2])
                idx_sb.append(it)
                locd_sb.append(lt)

            h1T = [hTp.tile([F, G * P], bf16, name=f"h1T_{g}", tag=f"hT{g}")
                   for g in range(NG)]

            def stage_a(layer, W_sb, souts_sb, g_dst, chunks):
                for c in chunks:
                    if layer == 1:
                        lhs_ap = xT_sb[:, c * P:(c + 1) * P]
                    else:
                        lhs_ap = h1T[c // G][:, (c % G) * P:(c % G + 1) * P]
                    pa = psA.tile([P, F], f32, name="pa", tag="pa")
                    nc.tensor.matmul(pa[:], lhs_ap, W_sb[:], start=True,
                                     stop=True)
                    gst = gstp.tile([P, F], bf16, name="gst", tag="gst")
                    nc.scalar.activation(
                        out=gst[:], in_=pa[:],
                        func=mybir.ActivationFunctionType.Copy,
                        scale=souts_sb[:, c:c + 1])
                    nc.sync.dma_start(g_dst[c * P:(c + 1) * P, :], gst[:])

            def one_hot(oh_ap, loc_tile, a0, n, parity):
                """oh[:, :n*P] = is_equal(iota+128*parity, locd[a0:a0+n])
                with a packed-pairs 4D AP so DVE runs in 2x_1p mode."""
                out4 = oh_ap.rearrange("p (t a b) -> p t a b", a=64, b=2)
                in0 = iota_sb[:, parity * WMAX * P:
                              parity * WMAX * P + n * P].rearrange(
                    "p (t a b) -> p t a b", a=64, b=2)
                sl = loc_tile[:, 2 * a0:2 * (a0 + n)]
                in1 = AP(sl.tensor, sl.offset,
                         [list(sl.ap[0]), [2, n], [0, 64], [1, 2]])
                nc.vector.tensor_tensor(out=out4, in0=in0, in1=in1,
                                        op=mybir.AluOpType.is_equal)

            def stage_b(layer, g_full, post_group):
                for g in range(NG):
                    gat = gatp.tile([P, Rmax * F], bf16, name="gat",
                                    tag="gat")
                    for c in range(NCH):
                        ncell = int(geom.cellcap[g, c])
                        if ncell == 0:
                            continue
                        S = int(geom.S[g, c])
                        sb = int(geom.sbase[g, c])   # cell subtile base
                        for s0 in range(0, S, MAXSUB):
                            s1 = min(s0 + MAXSUB, S)
                            n_idx = min(ncell - s0 * P, (s1 - s0) * P)
                            out_ap = gat[:, (sb + s0) * F:
                                         (sb + s1) * F].rearrange(
                                "p (s f) -> p s f", s=s1 - s0)
                            col0 = sb * 8 + s0 * 8
                            col1 = col0 + (-(-n_idx // 16))
                            nc.gpsimd.dma_gather(
                                out_ap=out_ap,
                                in_ap=g_full[c * CH:(c + 1) * CH, :],
                                idxs_ap=idx_sb[g][:, col0:col1],
                                num_idxs=n_idx,
                                num_idxs_reg=n_idx,
                                elem_size=F,
                                single_packet=False,
                            )
                    for i in range(G):
                        b = g * G + i
                        segs = [(int(geom.w0[g, c, i]), int(geom.w1[g, c, i]))
                                for c in range(NCH)
                                if geom.w1[g, c, i] > geom.w0[g, c, i]]
                        ntot = sum(a1 - a0 for a0, a1 in segs)
                        pb = psB.tile([F, P], f32, name="pb", tag="pb")
                        if ntot == 0:
                            nc.vector.memset(pb[:], 0.0)
                        ti = 0
                        for a0, a1 in segs:
                            n = a1 - a0
                            oh = ohp.tile([P, WMAX * P], bf16, name="oh",
                                          tag="oh")
                            one_hot(oh[:, :n * P], locd_sb[g], a0, n, i & 1)
                            for k in range(n):
                                # messages stationary: out[f, d] =
                                # sum_s gat[s, f] * oh[s, d]  (pre-transposed)
                                nc.tensor.matmul(
                                    pb[:],
                                    gat[:, (a0 + k) * F:(a0 + k + 1) * F],
                                    oh[:, k * P:(k + 1) * P],
                                    start=(ti == 0), stop=(ti == ntot - 1))
                                ti += 1
                        if layer == 1:
                            nc.scalar.activation(
                                out=h1T[g][:, i * P:(i + 1) * P], in_=pb[:],
                                func=mybir.ActivationFunctionType.Relu)
                        else:
                            hsl = hslp.tile([F, P], bf16, name="hsl",
                                            tag="hsl")
                            nc.scalar.activation(
                                out=hsl[:], in_=pb[:],
                                func=mybir.ActivationFunctionType.Relu)
                            pc = psC.tile([P, NCLS], f32, name="pc",
                                          tag="pc")
                            nc.tensor.matmul(pc[:], hsl[:], Wc_sb[:],
                                             start=True, stop=True)
                            o = outp.tile([P, NCLS], f32, name="o", tag="o")
                            nc.vector.scalar_tensor_tensor(
                                out=o[:], in0=pc[:],
                                scalar=sins_sb[:, b:b + 1], in1=bc_sb[:],
                                op0=mybir.AluOpType.mult,
                                op1=mybir.AluOpType.add)
                            nc.sync.dma_start(
                                logits[b * P:(b + 1) * P, :], o[:])
                    if post_group is not None:
                        post_group(g)

            def all_gather(g_full):
                if single_core_sim:
                    nc.sync.dma_start(g_full[:NPC, :], g_loc[:])
                else:
                    nc.gpsimd.collective_compute(
                        "AllGather", mybir.AluOpType.bypass,
                        replica_groups=[list(range(cfg.n_cores))],
                        ins=[g_loc[:]], outs=[g_full[:]])

            stage_a(1, W1_sb, souts1_sb, g_loc, range(NB))
            all_gather(g1_full)
            stage_b(1, g1_full,
                    post_group=lambda g: stage_a(
                        2, W2_sb, souts2_sb, g_loc,
                        range(g * G, (g + 1) * G)))
            all_gather(g2_full)
            stage_b(2, g2_full, None)

    nc.compile()
    return nc


def run(cfg: Cfg, features, src, dst, W1, b1, W2, b2, Wc, bc,
        trace=False, return_results=False):
    F, NPC, NPAD = cfg.in_feats, cfg.npc, cfg.npad
    n = cfg.n_nodes
    src = np.asarray(src).astype(np.int64)
    dst = np.asarray(dst).astype(np.int64)
    features = np.asarray(features, np.float32)
    # the epilogue folds s_in forward through the relu, which is exact
    # only for zero hidden biases (the spec pins b1 = b2 = 0)
    assert np.abs(np.asarray(b1)).max() == 0.0, "kernel requires b1 == 0"
    assert np.abs(np.asarray(b2)).max() == 0.0, "kernel requires b2 == 0"
    deg_out = np.bincount(src, minlength=NPAD).astype(np.float32)
    deg_in = np.bincount(dst, minlength=NPAD).astype(np.float32)
    s_out_old = 1.0 / np.sqrt(np.maximum(deg_out, 1.0))
    s_in_old = 1.0 / np.sqrt(np.maximum(deg_in, 1.0))

    geom, node_new, idx16, locd = preprocess(cfg, src, dst)

    x_new = np.zeros((NPAD, F), np.float32)
    x_new[node_new[:n]] = features
    s_out = np.ones(NPAD, np.float32)
    s_out[node_new] = s_out_old
    s_in = np.ones(NPAD, np.float32)
    s_in[node_new] = s_in_old
    xT_full = np.ascontiguousarray(x_new.T).astype(np_bf16)

    iota_pat = np.tile(np.arange(P, dtype=np.float32), geom.WMAX)
    iota_np = np.concatenate([iota_pat, iota_pat + P]).astype(np_bf16)
    iota_np = np.tile(iota_np, (P, 1))
    bc_b = np.tile(np.asarray(bc, np.float32)[None, :], (P, 1))

    in_maps = []
    for m in range(cfg.n_cores):
        sl = slice(m * NPC, (m + 1) * NPC)
        in_maps.append({
            "xT": np.ascontiguousarray(xT_full[:, sl]),
            "W1": np.asarray(W1, np.float32).astype(np_bf16),
            "W2": np.asarray(W2, np.float32).astype(np_bf16),
            "Wc": np.asarray(Wc, np.float32).astype(np_bf16),
            "bc": bc_b,
            "souts1": np.ascontiguousarray(
                s_out[sl].reshape(cfg.nb, P).T),
            "souts2": np.ascontiguousarray(
                (s_out[sl] * s_in[sl]).reshape(cfg.nb, P).T),
            "sins": np.ascontiguousarray(
                s_in[sl].reshape(cfg.nb, P).T),
            "idx16": idx16[m],
            "locm": locd[m],
            "iota_w": iota_np,
        })

    nc = build_program(cfg, geom)
    last_err = None
    for _attempt in range(3):
        try:
            res = run_bass_kernel_spmd(nc, in_maps, list(range(cfg.n_cores)),
                                       trace=trace)
            break
        except Exception as e:  # transient axon worker hiccups
            last_err = e
    else:
        raise last_err
    out_new = np.concatenate([r["logits"] for r in res.results], axis=0)
    out = out_new[node_new[:n]].astype(np.float32)
    if return_results:
        return out, res
    return out


def kernel(features, src, dst, W1, b1, W2, b2, Wc, bc):
    return run(CFG, features, src, dst, W1, b1, W2, b2, Wc, bc)


# revision 30
# speedup vs baseline: 1.1998x; 1.1140x over previous
"""Trainium2 Bass kernel for a 2-layer GCN (DGL GraphConv, norm='both').

Reference computation (per layer):
    h = relu( deg_in^-0.5 * segment_sum( ((x * deg_out^-0.5) @ W)[src], dst ) + b )
then logits = h2 @ Wc + bc.

Distribution: nodes are relabeled into 128-wide blocks, blocks are
load-balanced across the 8 NeuronCores (snake assignment by edge count),
giving every core an equal, structurally identical workload (SPMD: one
program, per-core data). Per layer:
  stage A: each core computes g = (x @ W) * s_out for its node shard
           (bf16 message table, 256B rows)
  AllGather: g shards -> full g table in every core's DRAM
  stage B: blocks are processed in groups of 7; slots are pooled per
    (group, chunk) cell with exact per-block capacities (no 128-rounding
    of block intervals), one dma_gather per cell; the per-block
    segment-sum is messages^T x one-hot matmuls accumulated in PSUM
    (messages stationary, so the result lands pre-transposed [F, P]),
    with block membership encoded in loc via a parity offset (adjacent
    blocks alternate +0/+128) so boundary subtiles shared by two blocks
    stay exact; epilogue is a single relu (b1 = b2 = 0, so
    relu(s_in * agg) = s_in * relu(agg) and the s_in scale folds into
    the NEXT stage-A per-node scale: souts2 = s_out * s_in).
Layer 2's epilogue is fused with the classifier:
    logits = (relu(agg2) @ Wc) * s_in2 + bc,
written per shard; the host reassembles and inverse-permutes.

All index preprocessing (degree counts, edge sorting/padding, relabeling)
is host-side numpy on integer graph structure; float math is on device.
"""
import math
from dataclasses import dataclass

import numpy as np

import concourse.bacc as bacc
import concourse.mybir as mybir
import concourse.tile as tile
from concourse.ap import AP
from concourse.bass_utils import run_bass_kernel_spmd

f32 = mybir.dt.float32
bf16 = mybir.dt.bfloat16
i16 = mybir.dt.int16

P = 128  # partitions / node block size

# numpy view of bfloat16 for host-side constant/input arrays
import ml_dtypes  # noqa: E402  (ships with jax)

np_bf16 = ml_dtypes.bfloat16


@dataclass
class Cfg:
    n_nodes: int = 100000
    in_feats: int = 128
    num_classes: int = 4
    n_cores: int = 8
    nb: int = 98          # node blocks per core
    chunk: int = 25088    # gather sub-table rows (int16-addressable)
    group: int = 7        # blocks per gather group

    @property
    def npc(self):        # nodes per core
        return self.nb * P

    @property
    def npad(self):       # padded node count
        return self.n_cores * self.npc

    @property
    def n_chunks(self):
        return math.ceil(self.npad / self.chunk)

    @property
    def n_groups(self):
        return math.ceil(self.nb / self.group)


CFG = Cfg()

MAXSUB = 60  # max subtiles per dma_gather call


class Geometry:
    """Static slot layout: per (group, chunk) cell, blocks get exact
    (cross-core max) capacities packed back-to-back; cells are
    subtile-aligned. Identical across cores."""

    def __init__(self, cfg: Cfg, bcap: np.ndarray):
        NG, NCH, G = cfg.n_groups, cfg.n_chunks, cfg.group
        self.bcap = bcap                      # [NG, NCH, G]
        self.cellcap = bcap.sum(axis=2)       # [NG, NCH]
        self.S = -(-self.cellcap // P)        # subtiles per cell
        # cell subtile base within group (cumulative over chunks)
        self.sbase = np.zeros((NG, NCH), np.int64)
        self.sbase[:, 1:] = np.cumsum(self.S, axis=1)[:, :-1]
        self.Bbase = self.sbase * P           # cell slot base
        self.Rg = self.S.sum(axis=1)          # [NG] subtiles per group
        self.Rmax = int(self.Rg.max())
        # block slot intervals within cell
        self.off = np.zeros((NG, NCH, G), np.int64)
        self.off[:, :, 1:] = np.cumsum(bcap, axis=2)[:, :, :-1]
        a = self.Bbase[:, :, None] + self.off           # incl. group base
        e = a + bcap
        self.w0 = a // P                      # window subtile start
        self.w1 = -(-e // P)                  # window subtile end (excl)
        self.w1 = np.where(bcap > 0, self.w1, self.w0)
        self.WMAX = int((self.w1 - self.w0).max())


def preprocess(cfg: Cfg, src: np.ndarray, dst: np.ndarray):
    """Relabel nodes, sort edges into group-pooled gather metadata.

    Returns (geom, node_new, idx16, locd):
      idx16[m]: [NG, P, Rmax*8] int16 indices (16-wrapped, 8x repl)
      locd[m]:  [NG, P, Rmax*2] bf16 parity-encoded local dst
                (pairwise-duplicated for the packed-last-dim DVE mode),
                1000 for pads
    Group slot j -> partition j%128, group subtile j//128.
    """
    ncores, nb, nch, CH = cfg.n_cores, cfg.nb, cfg.n_chunks, cfg.chunk
    G, NG = cfg.group, cfg.n_groups
    n_blocks = ncores * nb

    # block load balancing: snake-assign blocks by edge count
    blk_tot = np.bincount(dst >> 7, minlength=n_blocks)
    order = np.argsort(-blk_tot, kind="stable")
    rank = np.arange(n_blocks)
    lane = rank % ncores
    rev = (rank // ncores) % 2 == 1
    core_of_rank = np.where(rev, ncores - 1 - lane, lane)
    core_of_old = np.empty(n_blocks, np.int64)
    pos_of_old = np.empty(n_blocks, np.int64)
    core_of_old[order] = core_of_rank
    pos_of_old[order] = rank // ncores
    new_blk_of_old = core_of_old * nb + pos_of_old
    node_ar = np.arange(cfg.npad, dtype=np.int64)
    node_new = new_blk_of_old[node_ar >> 7] * P + (node_ar & 127)

    src_n = node_new[src]
    dst_n = node_new[dst]

    blk = dst_n >> 7
    m_arr = blk // nb
    b_arr = blk % nb
    g_arr = b_arr // G
    i_arr = b_arr % G
    c_arr = src_n // CH

    key = ((m_arr * NG + g_arr) * nch + c_arr) * G + i_arr
    counts = np.bincount(key, minlength=ncores * NG * nch * G).reshape(
        ncores, NG, nch, G)
    # exact per-block capacity = cross-core max; floor 128 so a subtile
    # never spans more than two (parity-distinct) block intervals
    bcap = np.maximum(counts.max(axis=0), P)
    geom = Geometry(cfg, bcap)

    # per-edge slot within its group's slot space
    perm = np.lexsort((src_n, i_arr, c_arr, g_arr, m_arr))
    src_s = src_n[perm]
    m_s = m_arr[perm]
    g_s = g_arr[perm]
    c_s = c_arr[perm]
    i_s = i_arr[perm]
    key_s = key[perm]
    ecum = np.zeros(ncores * NG * nch * G + 1, np.int64)
    np.cumsum(counts.reshape(-1), out=ecum[1:])
    within = np.arange(len(src_s)) - ecum[key_s]
    slot = geom.Bbase[g_s, c_s] + geom.off[g_s, c_s, i_s] + within

    Rmax = geom.Rmax
    locv = ((dst_n[perm] & 127) + P * (i_s & 1)).astype(np.float32)
    loc = np.full((ncores, NG, P, Rmax), 1000.0, np_bf16)
    t_arr = slot // P
    p_arr = slot % P
    loc[m_s, g_s, p_arr, t_arr] = locv.astype(np_bf16)
    locd = np.repeat(loc, 2, axis=3)  # pairwise duplicate along slots

    val = (src_s - c_s * CH).astype(np.int16)
    c16 = slot // 16
    r16 = slot % 16
    flat = np.zeros((ncores, NG, 16, Rmax * 8), np.int16)
    flat[m_s, g_s, r16, c16] = val
    idx16 = np.tile(flat, (1, 1, 8, 1))
    return geom, node_new, idx16, locd


def build_program(cfg: Cfg, geom: Geometry, single_core_sim=False):
    F = cfg.in_feats
    NB, NPC, NPAD = cfg.nb, cfg.npc, cfg.npad
    NCH, CH, NG, G = cfg.n_chunks, cfg.chunk, cfg.n_groups, cfg.group
    NCLS = cfg.num_classes
    Rmax, WMAX = geom.Rmax, geom.WMAX

    n_dev = 1 if single_core_sim else cfg.n_cores
    nc = bacc.Bacc("TRN2", target_bir_lowering=False, debug=False,
                   num_devices=n_dev)

    xg = nc.declare_dram_parameter("xg", [NPAD, F], bf16, isOutput=False)
    W1 = nc.declare_dram_parameter("W1", [F, F], bf16, isOutput=False)
    W2 = nc.declare_dram_parameter("W2", [F, F], bf16, isOutput=False)
    Wc = nc.declare_dram_parameter("Wc", [F, NCLS], bf16, isOutput=False)
    bc = nc.declare_dram_parameter("bc", [P, NCLS], f32, isOutput=False)
    souts2 = nc.declare_dram_parameter("souts2", [P, NB], f32, isOutput=False)
    sins = nc.declare_dram_parameter("sins", [P, NB], f32, isOutput=False)
    idx16 = nc.declare_dram_parameter("idx16", [NG, P, Rmax * 8], i16,
                                      isOutput=False)
    locm = nc.declare_dram_parameter("locm", [NG, P, Rmax * 2], bf16,
                                     isOutput=False)
    iota_w = nc.declare_dram_parameter("iota_w", [P, 2 * WMAX * P], bf16,
                                       isOutput=False)
    logits = nc.declare_dram_parameter("logits", [NPC, NCLS], f32,
                                       isOutput=True)

    with tile.TileContext(nc) as tc:
        with (
            tc.tile_pool(name="dram", bufs=1, space="DRAM") as dram,
            tc.tile_pool(name="consts", bufs=1) as consts,
            tc.tile_pool(name="gst", bufs=3) as gstp,
            tc.tile_pool(name="gat", bufs=14) as gatp,
            tc.tile_pool(name="oh", bufs=6) as ohp,
            tc.tile_pool(name="agx", bufs=3) as agxp,
            tc.tile_pool(name="hsl", bufs=3) as hslp,
            tc.tile_pool(name="out", bufs=3) as outp,
            tc.tile_pool(name="psB", bufs=5, space="PSUM") as psB,
            tc.tile_pool(name="psH1", bufs=1, space="PSUM") as psH1,
            tc.tile_pool(name="psH2", bufs=1, space="PSUM") as psH2,
            tc.tile_pool(name="psC", bufs=1, space="PSUM") as psC,
        ):
            # layer-2 message table: single bf16 per node row, 256B rows
            # (layer 1 gathers straight from the host-staged xg table)
            g_loc = dram.tile([NPC, F], bf16, name="g_loc")
            g2_full = dram.tile([NPAD, F], bf16, addr_space="Shared",
                                name="g2_full")

            W1_sb = consts.tile([F, F], bf16, name="W1_sb")
            nc.sync.dma_start(W1_sb[:], W1[:])
            W2_sb = consts.tile([F, F], bf16, name="W2_sb")
            nc.sync.dma_start(W2_sb[:], W2[:])
            Wc_sb = consts.tile([F, NCLS], bf16, name="Wc_sb")
            nc.sync.dma_start(Wc_sb[:], Wc[:])
            bc_sb = consts.tile([P, NCLS], f32, name="bc_sb")
            nc.sync.dma_start(bc_sb[:], bc[:])
            iota_sb = consts.tile([P, 2 * WMAX * P], bf16, name="iota_sb")
            nc.sync.dma_start(iota_sb[:], iota_w[:])
            souts2_sb = consts.tile([P, NB], f32, name="souts2_sb")
            nc.sync.dma_start(souts2_sb[:], souts2[:])
            sins_sb = consts.tile([P, NB], f32, name="sins_sb")
            nc.sync.dma_start(sins_sb[:], sins[:])

            # SBUF-resident per-group metadata (shared by both layers)
            idx_sb = []
            locd_sb = []
            for g in range(NG):
                Rg = int(geom.Rg[g])
                it = consts.tile([P, Rmax * 8], i16, name=f"idx_sb{g}")
                nc.sync.dma_start(it[:, :Rg * 8], idx16[g, :, :Rg * 8])
                lt = consts.tile([P, Rmax * 2], bf16, name=f"locd_sb{g}")
                nc.sync.dma_start(lt[:, :Rg * 2], locm[g, :, :Rg * 2])
                idx_sb.append(it)
                locd_sb.append(lt)

            SMAX = int(geom.S.max())

            def one_hot(oh_ap, loc_tile, a0, n, parity):
                """oh[:, :n*P] = is_equal(iota+128*parity, locd[a0:a0+n])
                with a packed-pairs 4D AP so DVE runs in 2x_1p mode."""
                out4 = oh_ap.rearrange("p (t a b) -> p t a b", a=64, b=2)
                in0 = iota_sb[:, parity * WMAX * P:
                              parity * WMAX * P + n * P].rearrange(
                    "p (t a b) -> p t a b", a=64, b=2)
                sl = loc_tile[:, 2 * a0:2 * (a0 + n)]
                in1 = AP(sl.tensor, sl.offset,
                         [list(sl.ap[0]), [2, n], [0, 64], [1, 2]])
                nc.vector.tensor_tensor(out=out4, in0=in0, in1=in1,
                                        op=mybir.AluOpType.is_equal)

            def stage_b(layer, g_full, post_group):
                for g in range(NG):
                    gats = {}
                    for c in range(NCH):
                        ncell = int(geom.cellcap[g, c])
                        if ncell == 0:
                            continue
                        S = int(geom.S[g, c])
                        sb = int(geom.sbase[g, c])   # cell subtile base
                        gat = gatp.tile([P, SMAX * F], bf16, name="gat",
                                        tag="gat")
                        gats[c] = gat
                        for s0 in range(0, S, MAXSUB):
                            s1 = min(s0 + MAXSUB, S)
                            n_idx = min(ncell - s0 * P, (s1 - s0) * P)
                            out_ap = gat[:, s0 * F:s1 * F].rearrange(
                                "p (s f) -> p s f", s=s1 - s0)
                            col0 = sb * 8 + s0 * 8
                            col1 = col0 + (-(-n_idx // 16))
                            nc.gpsimd.dma_gather(
                                out_ap=out_ap,
                                in_ap=g_full[c * CH:(c + 1) * CH, :],
                                idxs_ap=idx_sb[g][:, col0:col1],
                                num_idxs=n_idx,
                                num_idxs_reg=n_idx,
                                elem_size=F,
                                single_packet=False,
                            )
                    for i in range(G):
                        b = g * G + i
                        segs = [(c, int(geom.w0[g, c, i]),
                                 int(geom.w1[g, c, i]))
                                for c in range(NCH)
                                if geom.w1[g, c, i] > geom.w0[g, c, i]]
                        ntot = sum(a1 - a0 for _, a0, a1 in segs)
                        pb = psB.tile([F, P], f32, name="pb", tag="pb")
                        if ntot == 0:
                            nc.vector.memset(pb[:], 0.0)
                        ti = 0
                        for c, a0, a1 in segs:
                            n = a1 - a0
                            sb = int(geom.sbase[g, c])
                            gat = gats[c]
                            oh = ohp.tile([P, WMAX * P], bf16, name="oh",
                                          tag="oh")
                            one_hot(oh[:, :n * P], locd_sb[g], a0, n, i & 1)
                            for k in range(n):
                                # messages stationary: out[f, d] =
                                # sum_s gat[s, f] * oh[s, d]  (pre-transposed)
                                kc = a0 - sb + k
                                nc.tensor.matmul(
                                    pb[:],
                                    gat[:, kc * F:(kc + 1) * F],
                                    oh[:, k * P:(k + 1) * P],
                                    start=(ti == 0), stop=(ti == ntot - 1))
                                ti += 1
                        agx = agxp.tile([F, P], bf16, name="agx",
                                        tag="agx")
                        nc.scalar.activation(
                            out=agx[:], in_=pb[:],
                            func=mybir.ActivationFunctionType.Copy)
                        if layer == 1:
                            # t2 row = s2 * relu(aggX @ W1), node-major:
                            # matmul(lhsT=agx [F,P], rhs=W1) -> [P, F]
                            ph = psH1.tile([P, F], f32, name="ph", tag="ph")
                            nc.tensor.matmul(ph[:], agx[:], W1_sb[:],
                                             start=True, stop=True)
                            gst = gstp.tile([P, F], bf16, name="gst",
                                            tag="gst")
                            nc.scalar.activation(
                                out=gst[:], in_=ph[:],
                                func=mybir.ActivationFunctionType.Relu,
                                scale=souts2_sb[:, b:b + 1])
                            nc.sync.dma_start(
                                g_loc[b * P:(b + 1) * P, :], gst[:])
                        else:
                            # agg2 = (pb^T @ W2) kept transposed:
                            # matmul(lhsT=W2, rhs=agx) = (agg2)^T [F, P]
                            pw = psH2.tile([F, P], f32, name="pw", tag="pw")
                            nc.tensor.matmul(pw[:], W2_sb[:], agx[:],
                                             start=True, stop=True)
                            hsl = hslp.tile([F, P], bf16, name="hsl",
                                            tag="hsl")
                            nc.scalar.activation(
                                out=hsl[:], in_=pw[:],
                                func=mybir.ActivationFunctionType.Relu)
                            pc = psC.tile([P, NCLS], f32, name="pc",
                                          tag="pc")
                            nc.tensor.matmul(pc[:], hsl[:], Wc_sb[:],
                                             start=True, stop=True)
                            o = outp.tile([P, NCLS], f32, name="o", tag="o")
                            nc.vector.scalar_tensor_tensor(
                                out=o[:], in0=pc[:],
                                scalar=sins_sb[:, b:b + 1], in1=bc_sb[:],
                                op0=mybir.AluOpType.mult,
                                op1=mybir.AluOpType.add)
                            nc.sync.dma_start(
                                logits[b * P:(b + 1) * P, :], o[:])
                    if post_group is not None:
                        post_group(g)

            def all_gather(g_full):
                if single_core_sim:
                    nc.sync.dma_start(g_full[:NPC, :], g_loc[:])
                else:
                    nc.gpsimd.collective_compute(
                        "AllGather", mybir.AluOpType.bypass,
                        replica_groups=[list(range(cfg.n_cores))],
                        ins=[g_loc[:]], outs=[g_full[:]])

            stage_b(1, xg, None)
            all_gather(g2_full)
            stage_b(2, g2_full, None)

    nc.compile()
    return nc


def run(cfg: Cfg, features, src, dst, W1, b1, W2, b2, Wc, bc,
        trace=False, return_results=False):
    F, NPC, NPAD = cfg.in_feats, cfg.npc, cfg.npad
    n = cfg.n_nodes
    src = np.asarray(src).astype(np.int64)
    dst = np.asarray(dst).astype(np.int64)
    features = np.asarray(features, np.float32)
    # the epilogue folds s_in forward through the relu, which is exact
    # only for zero hidden biases (the spec pins b1 = b2 = 0)
    assert np.abs(np.asarray(b1)).max() == 0.0, "kernel requires b1 == 0"
    assert np.abs(np.asarray(b2)).max() == 0.0, "kernel requires b2 == 0"
    deg_out = np.bincount(src, minlength=NPAD).astype(np.float32)
    deg_in = np.bincount(dst, minlength=NPAD).astype(np.float32)
    s_out_old = 1.0 / np.sqrt(np.maximum(deg_out, 1.0))
    s_in_old = 1.0 / np.sqrt(np.maximum(deg_in, 1.0))

    geom, node_new, idx16, locd = preprocess(cfg, src, dst)

    x_new = np.zeros((NPAD, F), np.float32)
    x_new[node_new[:n]] = features
    s_out = np.ones(NPAD, np.float32)
    s_out[node_new] = s_out_old
    s_in = np.ones(NPAD, np.float32)
    s_in[node_new] = s_in_old
    xg_full = (x_new * s_out[:, None]).astype(np_bf16)

    iota_pat = np.tile(np.arange(P, dtype=np.float32), geom.WMAX)
    iota_np = np.concatenate([iota_pat, iota_pat + P]).astype(np_bf16)
    iota_np = np.tile(iota_np, (P, 1))
    bc_b = np.tile(np.asarray(bc, np.float32)[None, :], (P, 1))

    in_maps = []
    for m in range(cfg.n_cores):
        sl = slice(m * NPC, (m + 1) * NPC)
        in_maps.append({
            "xg": xg_full,
            "W1": np.asarray(W1, np.float32).astype(np_bf16),
            "W2": np.asarray(W2, np.float32).astype(np_bf16),
            "Wc": np.asarray(Wc, np.float32).astype(np_bf16),
            "bc": bc_b,
            "souts2": np.ascontiguousarray(
                (s_out[sl] * s_in[sl]).reshape(cfg.nb, P).T),
            "sins": np.ascontiguousarray(
                s_in[sl].reshape(cfg.nb, P).T),
            "idx16": idx16[m],
            "locm": locd[m],
            "iota_w": iota_np,
        })

    nc = build_program(cfg, geom)
    last_err = None
    for _attempt in range(3):
        try:
            res = run_bass_kernel_spmd(nc, in_maps, list(range(cfg.n_cores)),
                                       trace=trace)
            break
        except Exception as e:  # transient axon worker hiccups
            last_err = e
    else:
        raise last_err
    out_new = np.concatenate([r["logits"] for r in res.results], axis=0)
    out = out_new[node_new[:n]].astype(np.float32)
    if return_results:
        return out, res
    return out


def kernel(features, src, dst, W1, b1, W2, b2, Wc, bc):
    return run(CFG, features, src, dst, W1, b1, W2, b2, Wc, bc)
